# revision 22
# baseline (speedup 1.0000x reference)
"""Trainium2 Bass kernel for nn_DKEncoder (scatter_memory).

Math (per batch b, reformulated from the reference):
  qiL  = tanh(q0 @ WqL.T + bqL)                 (L in {2,1}, tiny)
  qpL  = qiL @ (WkvL / sqrt(100))               (fold the 1/sqrt(kd) scale)
  att2 = k2.flat(6144,100) @ qp2                (PE bf16, k2 host-transposed)
  a2   = masked-softmax_d(leaky_relu(att2))     (partition-group softmax)
  c2   = sum_d a2 * v2                          (PE bf16, block-diag selector)
  att1 = k1.flat(384,100) @ qp1
  a1   = masked-softmax_c(leaky_relu(att1))
  out  = sum_c a1 * concat([v1, c2], -1)        (PE bf16, accumulated selector)
  scatter rows to nonzero input_ent positions   (PE bf16, 0/1 gather matmul)

Sharding: pure data parallel, 4 batches per core across 8 cores.
All input-dependent data flows through DRAM parameters, so the program
is compiled once and reused for any inputs.

Perf notes:
- all big tensors stream as bf16 (halves HBM bytes and PE stationary loads)
- softmax divide on DVE (no Ln) so one activation table set covers
  Tanh/Exp/Copy -> no ACT_TABLE_LOAD stalls mid-kernel
- big DMAs spread across sync/scalar/vector/gpsimd queues
- bufs=4 pools let all per-batch loads prefetch up front
"""

import math
from contextlib import ExitStack

import ml_dtypes
import numpy as np

import concourse.bacc as bacc
import concourse.bass as bass
import concourse.mybir as mybir
import concourse.tile as tile

BF16NP = ml_dtypes.bfloat16

B, S, E, C, D, KD, QD = 32, 128, 24, 16, 16, 100, 768
NCORES = 8
BPC = B // NCORES          # batches per core
EC = E * C                 # 384 (e,c) rows
ROWS2 = EC * D             # 6144 (e,c,d) rows
NT2 = ROWS2 // 128         # 48 layer-0 tiles per batch
NT1 = EC // 128            # 3 layer-1 tiles per batch
NQ = QD // 128             # 6 q-chunks
OD = 2 * KD                # 200 output dim
F32 = mybir.dt.float32
BF16 = mybir.dt.bfloat16
AF = mybir.ActivationFunctionType
OP = mybir.AluOpType
FB = BF16

# packed-constants layout (bf16): name -> (rows, width)
CPACK_FIELDS = [
    ("q0t", 128, NQ * BPC),
    ("wq2t", 128, NQ * KD),
    ("wq1t", 128, NQ * KD),
    ("m24", 128, NT1 * E),
    ("sel16", 128, 8),
    ("wkv2", KD, KD),
    ("wkv1", KD, KD),
    ("ident", KD, KD),
    ("rep16", 8, 128),
    ("gmat", E, BPC * 128),
]
CPACK_W = sum(w for _, _, w in CPACK_FIELDS)
CPACK_OFF = {}
_off = 0
for _n, _r, _w in CPACK_FIELDS:
    CPACK_OFF[_n] = _off
    _off += _w


def build_nc() -> bass.Bass:
    nc = bacc.Bacc(None)
    p = lambda name, shape, out=False, dt=F32: nc.declare_dram_parameter(
        name, list(shape), dt, isOutput=out)

    k2t = p("k2t", [BPC, KD, ROWS2], dt=FB)  # per batch: k2 flat transposed
    v2r = p("v2r", [BPC, 128, NT2 * KD], dt=FB)  # per batch: v2 rows tiled
    k1t = p("k1t", [KD, BPC * EC], dt=FB)    # k1 flat transposed
    v1r = p("v1r", [128, BPC * NT1 * KD], dt=FB)  # v1 rows tiled
    cpack = p("cpack", [128, CPACK_W], dt=FB)     # small constants, bf16
    bqf = p("bqf", [KD, 2])                  # biases, f32
    out = p("out", [BPC, 128, OD], out=True, dt=FB)

    with tile.TileContext(nc) as tc, ExitStack() as ctx:
        _body(ctx, tc, nc, locals())
    nc.compile()
    return nc


def _body(ctx, tc, nc, t):
    consts = ctx.enter_context(tc.tile_pool(name="consts", bufs=1))

    cp = consts.tile([128, CPACK_W], FB, tag="cpack")
    nc.scalar.dma_start(cp[:], t["cpack"][:])
    bqf = consts.tile([KD, 2], F32, tag="bqf")
    nc.scalar.dma_start(bqf[:], t["bqf"][:])

    def cc(name):
        rows, w = next((r, w) for n, r, w in CPACK_FIELDS if n == name)
        o = CPACK_OFF[name]
        return cp[0:rows, o:o + w]

    q0t, wq2t, wq1t, m24, sel16 = cc("q0t"), cc("wq2t"), cc("wq1t"), cc("m24"), cc("sel16")
    wkv2, wkv1 = cc("wkv2"), cc("wkv1")
    ident, rep16, gmat = cc("ident"), cc("rep16"), cc("gmat")

    work = ctx.enter_context(tc.tile_pool(name="work", bufs=1))
    k2pool = ctx.enter_context(tc.tile_pool(name="k2t", bufs=4))
    v2pool = ctx.enter_context(tc.tile_pool(name="v2r", bufs=4))

    # issue every load up front; two HWDGE queues (sync + scalar), first-need
    # order: k2t[0]/cpack head their queues, halves interleave behind them
    k2tiles, v2tiles = [], []
    for j in range(BPC):
        k2tiles.append(k2pool.tile([KD, ROWS2], FB, tag="k2tile", name=f"k2tile{j}"))
        v2tiles.append(v2pool.tile([128, NT2 * KD], FB, tag="v2tile", name=f"v2tile{j}"))
    k1t = consts.tile([KD, BPC * EC], FB, tag="k1t")
    v1r = consts.tile([128, BPC * NT1 * KD], FB, tag="v1r")
    nc.sync.dma_start(k2tiles[0][:], t["k2t"][0, :, :])
    nc.scalar.dma_start(k2tiles[1][:], t["k2t"][1, :, :])
    nc.sync.dma_start(k1t[:], t["k1t"][:])
    nc.scalar.dma_start(v1r[:], t["v1r"][:])
    nc.sync.dma_start(v2tiles[0][:], t["v2r"][0, :, :])
    nc.scalar.dma_start(v2tiles[1][:], t["v2r"][1, :, :])
    nc.sync.dma_start(k2tiles[2][:], t["k2t"][2, :, :])
    nc.scalar.dma_start(k2tiles[3][:], t["k2t"][3, :, :])
    nc.sync.dma_start(v2tiles[2][:], t["v2r"][2, :, :])
    nc.scalar.dma_start(v2tiles[3][:], t["v2r"][3, :, :])

    # ---- Phase Q: qp2/qp1 [100, BPC+1] (zero pad col) ----
    qp = {}
    with tc.tile_pool(name="ps_q", bufs=2, space="PSUM") as ps_q:
        for lname, wqt, wkv, bqcol in (("qp2", wq2t, wkv2, 0), ("qp1", wq1t, wkv1, 1)):
            qtmp = ps_q.tile([KD, BPC], F32, tag="qtmp")
            for c in range(NQ):
                nc.tensor.matmul(
                    qtmp[:],
                    wqt[:, c * KD:(c + 1) * KD],
                    q0t[:, c * BPC:(c + 1) * BPC],
                    start=(c == 0), stop=(c == NQ - 1),
                )
            qi = work.tile([KD, BPC], FB, tag="qi")
            nc.scalar.activation(qi[:], qtmp[:], AF.Tanh,
                                 bias=bqf[:, bqcol:bqcol + 1], scale=1.0)
            qps = ps_q.tile([KD, BPC], F32, tag="qps")
            nc.tensor.matmul(qps[:], wkv[:], qi[:], start=True, stop=True)
            qsb = work.tile([KD, BPC + 1], FB, tag=lname)
            nc.vector.tensor_copy(qsb[:, 0:BPC], qps[:])
            nc.vector.memset(qsb[:, BPC:BPC + 1], 0.0)
            qp[lname] = qsb

    att_sel = work.tile([128, BPC * NT2 * 8], FB, tag="att_sel")
    sel24 = work.tile([128, BPC * NT1 * E], FB, tag="sel24")

    ps_att = ctx.enter_context(tc.tile_pool(name="ps_att", bufs=1, space="PSUM"))
    ps_sm = ctx.enter_context(tc.tile_pool(name="ps_sm", bufs=1, space="PSUM"))
    ps_c2 = ctx.enter_context(tc.tile_pool(name="ps_c2", bufs=2, space="PSUM"))
    ps_tp = ctx.enter_context(tc.tile_pool(name="ps_tp", bufs=1, space="PSUM"))
    ps_o1 = ctx.enter_context(tc.tile_pool(name="ps_o1", bufs=1, space="PSUM"))
    ps_g = ctx.enter_context(tc.tile_pool(name="ps_g", bufs=1, space="PSUM"))

    # magic constant for the DVE Newton reciprocal (no Ln -> one act table)
    I32 = mybir.dt.int32
    magic = work.tile([8, (BPC // 2) * NT2], I32, tag="magic")
    nc.vector.memset(magic[:], 0x7EF127EA)

    # group-of-16 partition softmax over a [128, nc2] range holding
    # [real, garbage] column pairs in PSUM; returns dense bf16 [128, ncols]
    def softmax(att_pair_view, ncols, tg):
        att_sb = work.tile([128, ncols], F32, tag=tg + "att")
        nc.scalar.activation(att_sb[:].unsqueeze(2), att_pair_view, AF.Copy)
        mask = work.tile([128, ncols], FB, tag=tg + "mask")
        nc.vector.tensor_scalar(mask[:], att_sb[:], 0.0, None, op0=OP.not_equal)
        lr = work.tile([128, ncols], F32, tag=tg + "lr")
        nc.vector.scalar_tensor_tensor(
            lr[:], att_sb[:], 0.01, att_sb[:], op0=OP.mult, op1=OP.max)
        ex = work.tile([128, ncols], FB, tag=tg + "ex")
        nc.scalar.activation(ex[:], lr[:], AF.Exp)
        exm = work.tile([128, ncols], FB, tag=tg + "exm")
        nc.vector.tensor_mul(exm[:], ex[:], mask[:])
        sums_ps = ps_sm.tile([8, ncols], F32, tag="sm_ps")
        nc.tensor.matmul(sums_ps[:], sel16[:], exm[:], start=True, stop=True)
        sums = work.tile([8, ncols], F32, tag=tg + "sumsb")
        nc.vector.tensor_scalar_add(sums[:], sums_ps[:], 1e-30)
        sums_n = work.tile([8, ncols], F32, tag=tg + "sumsn")
        nc.vector.tensor_scalar(
            sums_n[:], sums_ps[:], 1e-30, -1.0, op0=OP.add, op1=OP.mult)
        # rinv = 1/sums: magic-number seed + 2 Newton steps, all on DVE;
        # r' = (t+2)*r with t = (-x)*r keeps stt's (in0 op scalar) order safe
        i0 = work.tile([8, ncols], I32, tag=tg + "i0")
        nc.vector.tensor_sub(i0[:], magic[:, 0:ncols], sums[:].bitcast(I32))
        r0 = i0[:].bitcast(F32)
        t1 = work.tile([8, ncols], F32, tag=tg + "t1")
        nc.vector.tensor_mul(t1[:], sums_n[:], r0)
        r1 = work.tile([8, ncols], F32, tag=tg + "r1")
        nc.vector.scalar_tensor_tensor(
            r1[:], t1[:], 2.0, r0, op0=OP.add, op1=OP.mult)
        t2 = work.tile([8, ncols], F32, tag=tg + "t2")
        nc.vector.tensor_mul(t2[:], sums_n[:], r1[:])
        rinv = work.tile([8, ncols], FB, tag=tg + "rinv")
        nc.vector.scalar_tensor_tensor(
            rinv[:], t2[:], 2.0, r1[:], op0=OP.add, op1=OP.mult)
        rrep_ps = ps_sm.tile([128, ncols], F32, tag="sm_ps")
        nc.tensor.matmul(rrep_ps[:], rep16[:], rinv[:], start=True, stop=True)
        attn = work.tile([128, ncols], F32, tag=tg + "attn")
        nc.vector.tensor_mul(attn[:], exm[:], rrep_ps[:])
        m2 = work.tile([128, ncols], FB, tag=tg + "m2")
        nc.vector.tensor_scalar(m2[:], attn[:], 1.0 / 16.0, None, op0=OP.not_equal)
        attf = work.tile([128, ncols], FB, tag=tg + "attf")
        nc.vector.tensor_mul(attf[:], attn[:], m2[:])
        return attf

    att2_ps = ps_att.tile([128, 2 * BPC * NT2], F32, tag="att2")
    att1_ps = ps_att.tile([128, 2 * BPC * NT1], F32, tag="att1")

    HALF = BPC // 2
    for h in range(2):
        js = range(h * HALF, (h + 1) * HALF)
        # ---- attention logits for this half ----
        for j in js:
            k2tile = k2tiles[j]
            for tt in range(NT2):
                col = 2 * (j * NT2 + tt)
                nc.tensor.matmul(
                    att2_ps[:, col:col + 2],
                    k2tile[:, tt * 128:(tt + 1) * 128],
                    qp["qp2"][:, j:j + 2],
                    start=True, stop=True,
                )
            for tt in range(NT1):
                col = 2 * (j * NT1 + tt)
                nc.tensor.matmul(
                    att1_ps[:, col:col + 2],
                    k1t[:, j * EC + tt * 128: j * EC + (tt + 1) * 128],
                    qp["qp1"][:, j:j + 2],
                    start=True, stop=True,
                )

        # ---- softmax for this half ----
        n2, n1 = HALF * NT2, HALF * NT1
        a2view = att2_ps[:].rearrange("p (c two) -> p c two", two=2)[
            :, h * n2:(h + 1) * n2, 0:1]
        a1view = att1_ps[:].rearrange("p (c two) -> p c two", two=2)[
            :, h * n1:(h + 1) * n1, 0:1]
        att2f = softmax(a2view, n2, "s2_")
        att1f = softmax(a1view, n1, "s1_")

        # selector builds (0-step broadcast dims; mask picks the diagonal)
        nc.vector.tensor_mul(
            att_sel[:, h * n2 * 8:(h + 1) * n2 * 8].rearrange(
                "p (c g) -> p c g", g=8),
            att2f[:].unsqueeze(2).broadcast_to([128, n2, 8]),
            sel16[:].unsqueeze(1).broadcast_to([128, n2, 8]),
        )
        nc.vector.tensor_mul(
            sel24[:, h * n1 * E:(h + 1) * n1 * E].rearrange(
                "p (j t e) -> p j t e", j=HALF, t=NT1),
            att1f[:].rearrange("p (j t) -> p j t", j=HALF).unsqueeze(3)
            .broadcast_to([128, HALF, NT1, E]),
            m24[:].rearrange("p (t e) -> p t e", t=NT1).unsqueeze(1)
            .broadcast_to([128, HALF, NT1, E]),
        )

        # ---- combined2 (transposed), layer 1, gather, store ----
        for j in js:
            v2tile = v2tiles[j]
            c2t_ps = ps_c2.tile([KD, EC], F32, tag="c2t")
            for tt in range(NT2):
                nc.tensor.matmul(
                    c2t_ps[:, tt * 8:(tt + 1) * 8],
                    v2tile[:, tt * KD:(tt + 1) * KD],
                    att_sel[:, (j * NT2 + tt) * 8:(j * NT2 + tt + 1) * 8],
                    start=True, stop=True,
                )
            c2t = work.tile([KD, EC], FB, tag="c2t_sb")
            nc.vector.tensor_copy(c2t[:], c2t_ps[:])

            vcat = work.tile([128, NT1 * OD], FB, tag="vcat")
            for tt in range(NT1):
                nc.vector.tensor_copy(
                    vcat[:, tt * OD: tt * OD + KD],
                    v1r[:, (j * NT1 + tt) * KD:(j * NT1 + tt + 1) * KD],
                )
                tp_ps = ps_tp.tile([128, KD], FB, tag="tp")
                nc.tensor.transpose(tp_ps[:], c2t[:, tt * 128:(tt + 1) * 128], ident[:])
                nc.vector.tensor_copy(vcat[:, tt * OD + KD:(tt + 1) * OD], tp_ps[:])

            out1_ps = ps_o1.tile([E, OD], F32, tag="out1")
            for tt in range(NT1):
                nc.tensor.matmul(
                    out1_ps[:],
                    sel24[:, (j * NT1 + tt) * E:(j * NT1 + tt + 1) * E],
                    vcat[:, tt * OD:(tt + 1) * OD],
                    start=(tt == 0), stop=(tt == NT1 - 1),
                )
            table = work.tile([E, OD], FB, tag="table")
            nc.vector.tensor_copy(table[:], out1_ps[:])

            g_ps = ps_g.tile([128, OD], F32, tag="gath")
            nc.tensor.matmul(
                g_ps[:], gmat[:, j * 128:(j + 1) * 128], table[:],
                start=True, stop=True,
            )
            osb = work.tile([128, OD], FB, tag="osb")
            nc.vector.tensor_copy(osb[:], g_ps[:])
            nc.sync.dma_start(t["out"][j, :, :], osb[:])


def prep_inputs(inputs: dict) -> list[dict]:
    """Split full inputs into per-core input maps (host-side relayout only)."""
    q = np.ascontiguousarray(inputs["q"][:, 0, :], dtype=np.float32)      # [B, 768]
    k1 = np.asarray(inputs["k1"], dtype=np.float32)
    v1 = np.asarray(inputs["v1"], dtype=np.float32)
    k2 = np.asarray(inputs["k2"], dtype=np.float32)
    v2 = np.asarray(inputs["v2"], dtype=np.float32)
    ent = np.asarray(inputs["input_ent"])

    scale = np.float32(1.0 / math.sqrt(KD))
    wkv2 = np.asarray(inputs["Wkv2"], np.float32) * scale
    wkv1 = np.asarray(inputs["Wkv1"], np.float32) * scale
    wq2t = (np.asarray(inputs["Wq2"], np.float32).T.reshape(NQ, 128, KD)
            .transpose(1, 0, 2).reshape(128, NQ * KD))
    wq1t = (np.asarray(inputs["Wq1"], np.float32).T.reshape(NQ, 128, KD)
            .transpose(1, 0, 2).reshape(128, NQ * KD))
    bqf = np.stack([np.asarray(inputs["bq2"], np.float32),
                    np.asarray(inputs["bq1"], np.float32)], axis=1)  # [KD, 2]

    pp = np.arange(128)
    sel16 = (pp[:, None] // 16 == np.arange(8)[None, :]).astype(np.float32)
    rep16 = np.ascontiguousarray(sel16.T)
    te = np.arange(NT1 * E)
    m24 = (te[None, :] % E == 8 * (te[None, :] // E) + pp[:, None] // 16).astype(np.float32)
    ident = np.eye(KD, dtype=np.float32)

    mask = ent != 0
    rank = np.cumsum(mask, axis=1) - 1

    base = {"q0t": None, "wq2t": wq2t, "wq1t": wq1t, "m24": m24,
            "sel16": sel16, "wkv2": wkv2, "wkv1": wkv1,
            "ident": ident, "rep16": rep16, "gmat": None}

    maps = []
    for i in range(NCORES):
        bs = slice(i * BPC, (i + 1) * BPC)
        k2c, v2c = k2[bs], v2[bs]
        k1c, v1c = k1[bs], v1[bs]
        k2tc = np.ascontiguousarray(
            k2c.reshape(BPC, ROWS2, KD).transpose(0, 2, 1)).astype(BF16NP)
        v2rc = np.ascontiguousarray(
            v2c.reshape(BPC, NT2, 128, KD).transpose(0, 2, 1, 3)
            .reshape(BPC, 128, NT2 * KD)).astype(BF16NP)
        k1tc = np.ascontiguousarray(
            k1c.reshape(BPC, EC, KD).transpose(2, 0, 1)
            .reshape(KD, BPC * EC)).astype(BF16NP)
        v1rc = np.ascontiguousarray(
            v1c.reshape(BPC, NT1, 128, KD).transpose(2, 0, 1, 3)
            .reshape(128, BPC * NT1 * KD)).astype(BF16NP)
        q0tc = (q[bs].T.reshape(NQ, 128, BPC).transpose(1, 0, 2)
                .reshape(128, NQ * BPC))
        gm = np.zeros((E, BPC * 128), np.float32)
        for j in range(BPC):
            b = i * BPC + j
            for s in range(S):
                if mask[b, s]:
                    gm[rank[b, s], j * 128 + s] = 1.0

        cpk = np.zeros((128, CPACK_W), np.float32)
        vals = dict(base)
        vals["q0t"] = q0tc
        vals["gmat"] = gm
        for name, rows, w in CPACK_FIELDS:
            o = CPACK_OFF[name]
            cpk[0:rows, o:o + w] = vals[name]

        maps.append({
            "k2t": k2tc, "v2r": v2rc, "k1t": k1tc, "v1r": v1rc,
            "cpack": cpk.astype(BF16NP), "bqf": bqf,
        })
    return maps


_NC_CACHE = {}


def kernel(**inputs) -> np.ndarray:
    from concourse.bass_utils import run_bass_kernel_spmd

    if "nc" not in _NC_CACHE:
        _NC_CACHE["nc"] = build_nc()
    nc = _NC_CACHE["nc"]
    maps = prep_inputs(inputs)
    res = run_bass_kernel_spmd(nc, maps, list(range(NCORES))).results
    out = np.concatenate([np.asarray(res[i]["out"], dtype=np.float32)
                          for i in range(NCORES)], axis=0)
    return np.ascontiguousarray(out.reshape(B, S, OD))


# revision 23
# speedup vs baseline: 1.1475x; 1.1475x over previous
"""Trainium2 Bass kernel for nn_DKEncoder (scatter_memory).

Math (per batch b, reformulated from the reference):
  qiL  = tanh(q0 @ WqL.T + bqL)                 (L in {2,1}, tiny)
  qpL  = qiL @ (WkvL / sqrt(100))               (fold the 1/sqrt(kd) scale)
  att2 = k2.flat(6144,100) @ qp2                (PE bf16, k2 host-transposed)
  a2   = masked-softmax_d(leaky_relu(att2))     (partition-group softmax)
  c2   = sum_d a2 * v2                          (PE bf16, block-diag selector)
  att1 = k1.flat(384,100) @ qp1
  a1   = masked-softmax_c(leaky_relu(att1))
  out  = sum_c a1 * concat([v1, c2], -1)        (PE bf16, accumulated selector)
  scatter rows to nonzero input_ent positions   (PE bf16, 0/1 gather matmul)

Sharding: pure data parallel, 4 batches per core across 8 cores.
All input-dependent data flows through DRAM parameters, so the program
is compiled once and reused for any inputs.

Perf notes:
- all big tensors stream as bf16 (halves HBM bytes and PE stationary loads)
- softmax divide on DVE (no Ln) so one activation table set covers
  Tanh/Exp/Copy -> no ACT_TABLE_LOAD stalls mid-kernel
- big DMAs spread across sync/scalar/vector/gpsimd queues
- bufs=4 pools let all per-batch loads prefetch up front
"""

import math
from contextlib import ExitStack

import ml_dtypes
import numpy as np

import concourse.bacc as bacc
import concourse.bass as bass
import concourse.mybir as mybir
import concourse.tile as tile

BF16NP = ml_dtypes.bfloat16

B, S, E, C, D, KD, QD = 32, 128, 24, 16, 16, 100, 768
NCORES = 8
BPC = B // NCORES          # batches per core
EC = E * C                 # 384 (e,c) rows
ROWS2 = EC * D             # 6144 (e,c,d) rows
NT2 = ROWS2 // 128         # 48 layer-0 tiles per batch
NT1 = EC // 128            # 3 layer-1 tiles per batch
NQ = QD // 128             # 6 q-chunks
OD = 2 * KD                # 200 output dim
F32 = mybir.dt.float32
BF16 = mybir.dt.bfloat16
AF = mybir.ActivationFunctionType
OP = mybir.AluOpType
FB = BF16

# packed-constants layout (bf16): name -> (rows, width)
CPACK_FIELDS = [
    ("q0t", 128, NQ * BPC),
    ("wq2t", 128, NQ * KD),
    ("wq1t", 128, NQ * KD),
    ("m24", 128, NT1 * E),
    ("sel16", 128, 8),
    ("wkv2", KD, KD),
    ("wkv1", KD, KD),
    ("ident", KD, KD),
    ("rep16", 8, 128),
    ("gmat", E, BPC * 128),
]
CPACK_W = sum(w for _, _, w in CPACK_FIELDS)
CPACK_OFF = {}
_off = 0
for _n, _r, _w in CPACK_FIELDS:
    CPACK_OFF[_n] = _off
    _off += _w


def build_nc() -> bass.Bass:
    nc = bacc.Bacc(None)
    p = lambda name, shape, out=False, dt=F32: nc.declare_dram_parameter(
        name, list(shape), dt, isOutput=out)

    k2t = p("k2t", [BPC, KD, ROWS2], dt=FB)  # per batch: k2 flat transposed
    v2r = p("v2r", [BPC, 128, NT2 * KD], dt=FB)  # per batch: v2 rows tiled
    k1t = p("k1t", [KD, BPC * EC], dt=FB)    # k1 flat transposed
    v1r = p("v1r", [128, BPC * NT1 * KD], dt=FB)  # v1 rows tiled
    cpack = p("cpack", [128, CPACK_W], dt=FB)     # small constants, bf16
    bqf = p("bqf", [KD, 2])                  # biases, f32
    out = p("out", [BPC, 128, OD], out=True, dt=FB)

    with tile.TileContext(nc) as tc, ExitStack() as ctx:
        _body(ctx, tc, nc, locals())
    nc.compile()
    return nc


def _body(ctx, tc, nc, t):
    consts = ctx.enter_context(tc.tile_pool(name="consts", bufs=1))

    cp = consts.tile([128, CPACK_W], FB, tag="cpack")
    nc.scalar.dma_start(cp[:], t["cpack"][:])
    bqf = consts.tile([KD, 2], F32, tag="bqf")
    nc.scalar.dma_start(bqf[:], t["bqf"][:])

    def cc(name):
        rows, w = next((r, w) for n, r, w in CPACK_FIELDS if n == name)
        o = CPACK_OFF[name]
        return cp[0:rows, o:o + w]

    q0t, wq2t, wq1t, m24, sel16 = cc("q0t"), cc("wq2t"), cc("wq1t"), cc("m24"), cc("sel16")
    wkv2, wkv1 = cc("wkv2"), cc("wkv1")
    ident, rep16, gmat = cc("ident"), cc("rep16"), cc("gmat")

    work = ctx.enter_context(tc.tile_pool(name="work", bufs=1))
    k2pool = ctx.enter_context(tc.tile_pool(name="k2t", bufs=4))
    v2pool = ctx.enter_context(tc.tile_pool(name="v2r", bufs=4))

    # one load queue (sync) in first-need order — aggregate DMA bw is a
    # per-core ceiling, so extra queues don't help; keeping the scalar
    # queue to cpack/bqf only means activations never sit behind a
    # blocked dma_start in the scalar sequencer
    k2tiles, v2tiles = [], []
    for j in range(BPC):
        k2tiles.append(k2pool.tile([KD, ROWS2], FB, tag="k2tile", name=f"k2tile{j}"))
        v2tiles.append(v2pool.tile([128, NT2 * KD], FB, tag="v2tile", name=f"v2tile{j}"))
    k1t = consts.tile([KD, BPC * EC], FB, tag="k1t")
    v1r = consts.tile([128, BPC * NT1 * KD], FB, tag="v1r")
    nc.sync.dma_start(k2tiles[0][:], t["k2t"][0, :, :])
    nc.sync.dma_start(k1t[:], t["k1t"][:])
    nc.sync.dma_start(k2tiles[1][:], t["k2t"][1, :, :])
    nc.sync.dma_start(v1r[:], t["v1r"][:])
    nc.sync.dma_start(v2tiles[0][:], t["v2r"][0, :, :])
    nc.sync.dma_start(v2tiles[1][:], t["v2r"][1, :, :])
    nc.sync.dma_start(k2tiles[2][:], t["k2t"][2, :, :])
    nc.sync.dma_start(v2tiles[2][:], t["v2r"][2, :, :])
    nc.sync.dma_start(k2tiles[3][:], t["k2t"][3, :, :])
    nc.sync.dma_start(v2tiles[3][:], t["v2r"][3, :, :])

    # ---- Phase Q: qp2/qp1 [100, BPC+1] (zero pad col) ----
    qp = {}
    with tc.tile_pool(name="ps_q", bufs=2, space="PSUM") as ps_q:
        for lname, wqt, wkv, bqcol in (("qp2", wq2t, wkv2, 0), ("qp1", wq1t, wkv1, 1)):
            qtmp = ps_q.tile([KD, BPC], F32, tag="qtmp")
            for c in range(NQ):
                nc.tensor.matmul(
                    qtmp[:],
                    wqt[:, c * KD:(c + 1) * KD],
                    q0t[:, c * BPC:(c + 1) * BPC],
                    start=(c == 0), stop=(c == NQ - 1),
                )
            qi = work.tile([KD, BPC], FB, tag="qi")
            nc.scalar.activation(qi[:], qtmp[:], AF.Tanh,
                                 bias=bqf[:, bqcol:bqcol + 1], scale=1.0)
            qps = ps_q.tile([KD, BPC], F32, tag="qps")
            nc.tensor.matmul(qps[:], wkv[:], qi[:], start=True, stop=True)
            qsb = work.tile([KD, BPC + 1], FB, tag=lname)
            nc.vector.tensor_copy(qsb[:, 0:BPC], qps[:])
            nc.vector.memset(qsb[:, BPC:BPC + 1], 0.0)
            qp[lname] = qsb

    att_sel = work.tile([128, BPC * NT2 * 8], FB, tag="att_sel")
    sel24 = work.tile([128, BPC * NT1 * E], FB, tag="sel24")

    ps_att = ctx.enter_context(tc.tile_pool(name="ps_att", bufs=1, space="PSUM"))
    ps_sm = ctx.enter_context(tc.tile_pool(name="ps_sm", bufs=1, space="PSUM"))
    ps_c2 = ctx.enter_context(tc.tile_pool(name="ps_c2", bufs=2, space="PSUM"))
    ps_tp = ctx.enter_context(tc.tile_pool(name="ps_tp", bufs=1, space="PSUM"))
    ps_o1 = ctx.enter_context(tc.tile_pool(name="ps_o1", bufs=1, space="PSUM"))
    ps_g = ctx.enter_context(tc.tile_pool(name="ps_g", bufs=1, space="PSUM"))

    # magic constant for the DVE Newton reciprocal (no Ln -> one act table)
    I32 = mybir.dt.int32
    magic = work.tile([8, (BPC // 2) * NT2], I32, tag="magic")
    nc.vector.memset(magic[:], 0x7EF127EA)

    # group-of-16 partition softmax over a [128, nc2] range holding
    # [real, garbage] column pairs in PSUM; returns dense bf16 [128, ncols]
    def softmax(att_pair_view, ncols, tg):
        att_sb = work.tile([128, ncols], F32, tag=tg + "att")
        nc.scalar.activation(att_sb[:].unsqueeze(2), att_pair_view, AF.Copy)
        mask = work.tile([128, ncols], FB, tag=tg + "mask")
        nc.vector.tensor_scalar(mask[:], att_sb[:], 0.0, None, op0=OP.not_equal)
        lr = work.tile([128, ncols], F32, tag=tg + "lr")
        nc.vector.scalar_tensor_tensor(
            lr[:], att_sb[:], 0.01, att_sb[:], op0=OP.mult, op1=OP.max)
        ex = work.tile([128, ncols], FB, tag=tg + "ex")
        nc.scalar.activation(ex[:], lr[:], AF.Exp)
        exm = work.tile([128, ncols], FB, tag=tg + "exm")
        nc.vector.tensor_mul(exm[:], ex[:], mask[:])
        sums_ps = ps_sm.tile([8, ncols], F32, tag="sm_ps")
        nc.tensor.matmul(sums_ps[:], sel16[:], exm[:], start=True, stop=True)
        sums = work.tile([8, ncols], F32, tag=tg + "sumsb")
        nc.vector.tensor_scalar_add(sums[:], sums_ps[:], 1e-30)
        sums_n = work.tile([8, ncols], F32, tag=tg + "sumsn")
        nc.vector.tensor_scalar(
            sums_n[:], sums_ps[:], 1e-30, -1.0, op0=OP.add, op1=OP.mult)
        # rinv = 1/sums: magic-number seed + 2 Newton steps, all on DVE;
        # r' = (t+2)*r with t = (-x)*r keeps stt's (in0 op scalar) order safe
        i0 = work.tile([8, ncols], I32, tag=tg + "i0")
        nc.vector.tensor_sub(i0[:], magic[:, 0:ncols], sums[:].bitcast(I32))
        r0 = i0[:].bitcast(F32)
        t1 = work.tile([8, ncols], F32, tag=tg + "t1")
        nc.vector.tensor_mul(t1[:], sums_n[:], r0)
        r1 = work.tile([8, ncols], F32, tag=tg + "r1")
        nc.vector.scalar_tensor_tensor(
            r1[:], t1[:], 2.0, r0, op0=OP.add, op1=OP.mult)
        t2 = work.tile([8, ncols], F32, tag=tg + "t2")
        nc.vector.tensor_mul(t2[:], sums_n[:], r1[:])
        rinv = work.tile([8, ncols], FB, tag=tg + "rinv")
        nc.vector.scalar_tensor_tensor(
            rinv[:], t2[:], 2.0, r1[:], op0=OP.add, op1=OP.mult)
        rrep_ps = ps_sm.tile([128, ncols], F32, tag="sm_ps")
        nc.tensor.matmul(rrep_ps[:], rep16[:], rinv[:], start=True, stop=True)
        attn = work.tile([128, ncols], F32, tag=tg + "attn")
        nc.vector.tensor_mul(attn[:], exm[:], rrep_ps[:])
        m2 = work.tile([128, ncols], FB, tag=tg + "m2")
        nc.vector.tensor_scalar(m2[:], attn[:], 1.0 / 16.0, None, op0=OP.not_equal)
        attf = work.tile([128, ncols], FB, tag=tg + "attf")
        nc.vector.tensor_mul(attf[:], attn[:], m2[:])
        return attf

    att2_ps = ps_att.tile([128, 2 * BPC * NT2], F32, tag="att2")
    att1_ps = ps_att.tile([128, 2 * BPC * NT1], F32, tag="att1")

    HALF = BPC // 2
    for h in range(2):
        js = range(h * HALF, (h + 1) * HALF)
        # ---- attention logits for this half ----
        for j in js:
            k2tile = k2tiles[j]
            for tt in range(NT2):
                col = 2 * (j * NT2 + tt)
                nc.tensor.matmul(
                    att2_ps[:, col:col + 2],
                    k2tile[:, tt * 128:(tt + 1) * 128],
                    qp["qp2"][:, j:j + 2],
                    start=True, stop=True,
                )
            for tt in range(NT1):
                col = 2 * (j * NT1 + tt)
                nc.tensor.matmul(
                    att1_ps[:, col:col + 2],
                    k1t[:, j * EC + tt * 128: j * EC + (tt + 1) * 128],
                    qp["qp1"][:, j:j + 2],
                    start=True, stop=True,
                )

        # ---- softmax for this half ----
        n2, n1 = HALF * NT2, HALF * NT1
        a2view = att2_ps[:].rearrange("p (c two) -> p c two", two=2)[
            :, h * n2:(h + 1) * n2, 0:1]
        a1view = att1_ps[:].rearrange("p (c two) -> p c two", two=2)[
            :, h * n1:(h + 1) * n1, 0:1]
        att2f = softmax(a2view, n2, "s2_")
        att1f = softmax(a1view, n1, "s1_")

        # selector builds (0-step broadcast dims; mask picks the diagonal)
        nc.vector.tensor_mul(
            att_sel[:, h * n2 * 8:(h + 1) * n2 * 8].rearrange(
                "p (c g) -> p c g", g=8),
            att2f[:].unsqueeze(2).broadcast_to([128, n2, 8]),
            sel16[:].unsqueeze(1).broadcast_to([128, n2, 8]),
        )
        nc.vector.tensor_mul(
            sel24[:, h * n1 * E:(h + 1) * n1 * E].rearrange(
                "p (j t e) -> p j t e", j=HALF, t=NT1),
            att1f[:].rearrange("p (j t) -> p j t", j=HALF).unsqueeze(3)
            .broadcast_to([128, HALF, NT1, E]),
            m24[:].rearrange("p (t e) -> p t e", t=NT1).unsqueeze(1)
            .broadcast_to([128, HALF, NT1, E]),
        )

        # ---- combined2 (transposed), layer 1, gather, store ----
        for j in js:
            v2tile = v2tiles[j]
            c2t_ps = ps_c2.tile([KD, EC], F32, tag="c2t")
            for tt in range(NT2):
                nc.tensor.matmul(
                    c2t_ps[:, tt * 8:(tt + 1) * 8],
                    v2tile[:, tt * KD:(tt + 1) * KD],
                    att_sel[:, (j * NT2 + tt) * 8:(j * NT2 + tt + 1) * 8],
                    start=True, stop=True,
                )
            c2t = work.tile([KD, EC], FB, tag="c2t_sb")
            nc.vector.tensor_copy(c2t[:], c2t_ps[:])

            vcat = work.tile([128, NT1 * OD], FB, tag="vcat")
            for tt in range(NT1):
                nc.vector.tensor_copy(
                    vcat[:, tt * OD: tt * OD + KD],
                    v1r[:, (j * NT1 + tt) * KD:(j * NT1 + tt + 1) * KD],
                )
                tp_ps = ps_tp.tile([128, KD], FB, tag="tp")
                nc.tensor.transpose(tp_ps[:], c2t[:, tt * 128:(tt + 1) * 128], ident[:])
                nc.vector.tensor_copy(vcat[:, tt * OD + KD:(tt + 1) * OD], tp_ps[:])

            out1_ps = ps_o1.tile([E, OD], F32, tag="out1")
            for tt in range(NT1):
                nc.tensor.matmul(
                    out1_ps[:],
                    sel24[:, (j * NT1 + tt) * E:(j * NT1 + tt + 1) * E],
                    vcat[:, tt * OD:(tt + 1) * OD],
                    start=(tt == 0), stop=(tt == NT1 - 1),
                )
            table = work.tile([E, OD], FB, tag="table")
            nc.vector.tensor_copy(table[:], out1_ps[:])

            g_ps = ps_g.tile([128, OD], F32, tag="gath")
            nc.tensor.matmul(
                g_ps[:], gmat[:, j * 128:(j + 1) * 128], table[:],
                start=True, stop=True,
            )
            osb = work.tile([128, OD], FB, tag="osb")
            nc.vector.tensor_copy(osb[:], g_ps[:])
            nc.sync.dma_start(t["out"][j, :, :], osb[:])


def prep_inputs(inputs: dict) -> list[dict]:
    """Split full inputs into per-core input maps (host-side relayout only)."""
    q = np.ascontiguousarray(inputs["q"][:, 0, :], dtype=np.float32)      # [B, 768]
    k1 = np.asarray(inputs["k1"], dtype=np.float32)
    v1 = np.asarray(inputs["v1"], dtype=np.float32)
    k2 = np.asarray(inputs["k2"], dtype=np.float32)
    v2 = np.asarray(inputs["v2"], dtype=np.float32)
    ent = np.asarray(inputs["input_ent"])

    scale = np.float32(1.0 / math.sqrt(KD))
    wkv2 = np.asarray(inputs["Wkv2"], np.float32) * scale
    wkv1 = np.asarray(inputs["Wkv1"], np.float32) * scale
    wq2t = (np.asarray(inputs["Wq2"], np.float32).T.reshape(NQ, 128, KD)
            .transpose(1, 0, 2).reshape(128, NQ * KD))
    wq1t = (np.asarray(inputs["Wq1"], np.float32).T.reshape(NQ, 128, KD)
            .transpose(1, 0, 2).reshape(128, NQ * KD))
    bqf = np.stack([np.asarray(inputs["bq2"], np.float32),
                    np.asarray(inputs["bq1"], np.float32)], axis=1)  # [KD, 2]

    pp = np.arange(128)
    sel16 = (pp[:, None] // 16 == np.arange(8)[None, :]).astype(np.float32)
    rep16 = np.ascontiguousarray(sel16.T)
    te = np.arange(NT1 * E)
    m24 = (te[None, :] % E == 8 * (te[None, :] // E) + pp[:, None] // 16).astype(np.float32)
    ident = np.eye(KD, dtype=np.float32)

    mask = ent != 0
    rank = np.cumsum(mask, axis=1) - 1

    base = {"q0t": None, "wq2t": wq2t, "wq1t": wq1t, "m24": m24,
            "sel16": sel16, "wkv2": wkv2, "wkv1": wkv1,
            "ident": ident, "rep16": rep16, "gmat": None}

    maps = []
    for i in range(NCORES):
        bs = slice(i * BPC, (i + 1) * BPC)
        k2c, v2c = k2[bs], v2[bs]
        k1c, v1c = k1[bs], v1[bs]
        k2tc = np.ascontiguousarray(
            k2c.reshape(BPC, ROWS2, KD).transpose(0, 2, 1)).astype(BF16NP)
        v2rc = np.ascontiguousarray(
            v2c.reshape(BPC, NT2, 128, KD).transpose(0, 2, 1, 3)
            .reshape(BPC, 128, NT2 * KD)).astype(BF16NP)
        k1tc = np.ascontiguousarray(
            k1c.reshape(BPC, EC, KD).transpose(2, 0, 1)
            .reshape(KD, BPC * EC)).astype(BF16NP)
        v1rc = np.ascontiguousarray(
            v1c.reshape(BPC, NT1, 128, KD).transpose(2, 0, 1, 3)
            .reshape(128, BPC * NT1 * KD)).astype(BF16NP)
        q0tc = (q[bs].T.reshape(NQ, 128, BPC).transpose(1, 0, 2)
                .reshape(128, NQ * BPC))
        gm = np.zeros((E, BPC * 128), np.float32)
        for j in range(BPC):
            b = i * BPC + j
            for s in range(S):
                if mask[b, s]:
                    gm[rank[b, s], j * 128 + s] = 1.0

        cpk = np.zeros((128, CPACK_W), np.float32)
        vals = dict(base)
        vals["q0t"] = q0tc
        vals["gmat"] = gm
        for name, rows, w in CPACK_FIELDS:
            o = CPACK_OFF[name]
            cpk[0:rows, o:o + w] = vals[name]

        maps.append({
            "k2t": k2tc, "v2r": v2rc, "k1t": k1tc, "v1r": v1rc,
            "cpack": cpk.astype(BF16NP), "bqf": bqf,
        })
    return maps


_NC_CACHE = {}


def kernel(**inputs) -> np.ndarray:
    from concourse.bass_utils import run_bass_kernel_spmd

    if "nc" not in _NC_CACHE:
        _NC_CACHE["nc"] = build_nc()
    nc = _NC_CACHE["nc"]
    maps = prep_inputs(inputs)
    res = run_bass_kernel_spmd(nc, maps, list(range(NCORES))).results
    out = np.concatenate([np.asarray(res[i]["out"], dtype=np.float32)
                          for i in range(NCORES)], axis=0)
    return np.ascontiguousarray(out.reshape(B, S, OD))


# revision 28
# speedup vs baseline: 1.1647x; 1.0150x over previous
"""Trainium2 Bass kernel for nn_DKEncoder (scatter_memory).

Math (per batch b, reformulated from the reference):
  qiL  = tanh(q0 @ WqL.T + bqL)                 (L in {2,1}, tiny)
  qpL  = qiL @ (WkvL / sqrt(100))               (fold the 1/sqrt(kd) scale)
  att2 = k2.flat(6144,100) @ qp2                (PE bf16, k2 host-transposed)
  a2   = masked-softmax_d(leaky_relu(att2))     (partition-group softmax)
  c2   = sum_d a2 * v2                          (PE bf16, block-diag selector)
  att1 = k1.flat(384,100) @ qp1
  a1   = masked-softmax_c(leaky_relu(att1))
  out  = sum_c a1 * concat([v1, c2], -1)        (PE bf16, accumulated selector)
  scatter rows to nonzero input_ent positions   (PE bf16, 0/1 gather matmul)

Sharding: pure data parallel, 4 batches per core across 8 cores.
All input-dependent data flows through DRAM parameters, so the program
is compiled once and reused for any inputs.

Perf notes:
- all big tensors stream as bf16 (halves HBM bytes and PE stationary loads)
- softmax divide on DVE (no Ln) so one activation table set covers
  Tanh/Exp/Copy -> no ACT_TABLE_LOAD stalls mid-kernel
- big DMAs spread across sync/scalar/vector/gpsimd queues
- bufs=4 pools let all per-batch loads prefetch up front
"""

import math
from contextlib import ExitStack

import ml_dtypes
import numpy as np

import concourse.bacc as bacc
import concourse.bass as bass
import concourse.mybir as mybir
import concourse.tile as tile

BF16NP = ml_dtypes.bfloat16

B, S, E, C, D, KD, QD = 32, 128, 24, 16, 16, 100, 768
NCORES = 8
BPC = B // NCORES          # batches per core
EC = E * C                 # 384 (e,c) rows
ROWS2 = EC * D             # 6144 (e,c,d) rows
NT2 = ROWS2 // 128         # 48 layer-0 tiles per batch
NT1 = EC // 128            # 3 layer-1 tiles per batch
NQ = QD // 128             # 6 q-chunks
OD = 2 * KD                # 200 output dim
F32 = mybir.dt.float32
BF16 = mybir.dt.bfloat16
AF = mybir.ActivationFunctionType
OP = mybir.AluOpType
FB = BF16

# packed-constants layout (bf16): name -> (rows, width)
CPACK_FIELDS = [
    ("q0t", 128, NQ * BPC),
    ("wq2t", 128, NQ * KD),
    ("wq1t", 128, NQ * KD),
    ("m24", 128, NT1 * E),
    ("sel16", 128, 8),
    ("wkv2", KD, KD),
    ("wkv1", KD, KD),
    ("ident", KD, KD),
    ("rep16", 8, 128),
    ("gmat", E, BPC * 128),
]
CPACK_W = sum(w for _, _, w in CPACK_FIELDS)
CPACK_OFF = {}
_off = 0
for _n, _r, _w in CPACK_FIELDS:
    CPACK_OFF[_n] = _off
    _off += _w


def build_nc() -> bass.Bass:
    nc = bacc.Bacc(None)
    p = lambda name, shape, out=False, dt=F32: nc.declare_dram_parameter(
        name, list(shape), dt, isOutput=out)

    k2t = p("k2t", [BPC, KD, ROWS2], dt=FB)  # per batch: k2 flat transposed
    v2r = p("v2r", [BPC, 128, NT2 * KD], dt=FB)  # per batch: v2 rows tiled
    k1t = p("k1t", [KD, BPC * EC], dt=FB)    # k1 flat transposed
    v1r = p("v1r", [128, BPC * NT1 * KD], dt=FB)  # v1 rows tiled
    cpack = p("cpack", [128, CPACK_W], dt=FB)     # small constants, bf16
    bqf = p("bqf", [KD, 2])                  # biases, f32
    out = p("out", [BPC, 128, OD], out=True, dt=FB)

    with tile.TileContext(nc) as tc, ExitStack() as ctx:
        _body(ctx, tc, nc, locals())
    nc.compile()
    return nc


def _body(ctx, tc, nc, t):
    consts = ctx.enter_context(tc.tile_pool(name="consts", bufs=1))

    cp = consts.tile([128, CPACK_W], FB, tag="cpack")
    nc.scalar.dma_start(cp[:], t["cpack"][:])
    bqf = consts.tile([KD, 2], F32, tag="bqf")
    nc.scalar.dma_start(bqf[:], t["bqf"][:])

    def cc(name):
        rows, w = next((r, w) for n, r, w in CPACK_FIELDS if n == name)
        o = CPACK_OFF[name]
        return cp[0:rows, o:o + w]

    q0t, wq2t, wq1t, m24, sel16 = cc("q0t"), cc("wq2t"), cc("wq1t"), cc("m24"), cc("sel16")
    wkv2, wkv1 = cc("wkv2"), cc("wkv1")
    ident, rep16, gmat = cc("ident"), cc("rep16"), cc("gmat")

    work = ctx.enter_context(tc.tile_pool(name="work", bufs=1))
    k2pool = ctx.enter_context(tc.tile_pool(name="k2t", bufs=4))
    v2pool = ctx.enter_context(tc.tile_pool(name="v2r", bufs=4))

    # one load queue (sync) in first-need order — aggregate DMA bw is a
    # per-core ceiling, so extra queues don't help; keeping the scalar
    # queue to cpack/bqf only means activations never sit behind a
    # blocked dma_start in the scalar sequencer
    k2tiles, v2tiles = [], []
    for j in range(BPC):
        k2tiles.append(k2pool.tile([KD, ROWS2], FB, tag="k2tile", name=f"k2tile{j}"))
        v2tiles.append(v2pool.tile([128, NT2 * KD], FB, tag="v2tile", name=f"v2tile{j}"))
    k1t = consts.tile([KD, BPC * EC], FB, tag="k1t")
    v1r = consts.tile([128, BPC * NT1 * KD], FB, tag="v1r")
    nc.sync.dma_start(k2tiles[0][:], t["k2t"][0, :, :])
    nc.sync.dma_start(k1t[:], t["k1t"][:])
    nc.sync.dma_start(v1r[:], t["v1r"][:])
    nc.sync.dma_start(v2tiles[0][:], t["v2r"][0, :, :])
    nc.sync.dma_start(k2tiles[1][:], t["k2t"][1, :, :])
    nc.sync.dma_start(v2tiles[1][:], t["v2r"][1, :, :])
    nc.sync.dma_start(k2tiles[2][:], t["k2t"][2, :, :])
    nc.sync.dma_start(v2tiles[2][:], t["v2r"][2, :, :])
    nc.sync.dma_start(k2tiles[3][:], t["k2t"][3, :, :])
    nc.sync.dma_start(v2tiles[3][:], t["v2r"][3, :, :])

    # ---- Phase Q: qp2/qp1 [100, BPC+1] (zero pad col) ----
    qp = {}
    with tc.tile_pool(name="ps_q", bufs=2, space="PSUM") as ps_q:
        for lname, wqt, wkv, bqcol in (("qp2", wq2t, wkv2, 0), ("qp1", wq1t, wkv1, 1)):
            qtmp = ps_q.tile([KD, BPC], F32, tag="qtmp")
            for c in range(NQ):
                nc.tensor.matmul(
                    qtmp[:],
                    wqt[:, c * KD:(c + 1) * KD],
                    q0t[:, c * BPC:(c + 1) * BPC],
                    start=(c == 0), stop=(c == NQ - 1),
                )
            qi = work.tile([KD, BPC], FB, tag="qi")
            nc.scalar.activation(qi[:], qtmp[:], AF.Tanh,
                                 bias=bqf[:, bqcol:bqcol + 1], scale=1.0)
            qps = ps_q.tile([KD, BPC], F32, tag="qps")
            nc.tensor.matmul(qps[:], wkv[:], qi[:], start=True, stop=True)
            qsb = work.tile([KD, BPC + 1], FB, tag=lname)
            nc.vector.tensor_copy(qsb[:, 0:BPC], qps[:])
            nc.vector.memset(qsb[:, BPC:BPC + 1], 0.0)
            qp[lname] = qsb

    att_sel = work.tile([128, BPC * NT2 * 8], FB, tag="att_sel")
    sel24 = work.tile([128, BPC * NT1 * E], FB, tag="sel24")

    ps_att = ctx.enter_context(tc.tile_pool(name="ps_att", bufs=1, space="PSUM"))
    ps_sm = ctx.enter_context(tc.tile_pool(name="ps_sm", bufs=1, space="PSUM"))
    ps_c2 = ctx.enter_context(tc.tile_pool(name="ps_c2", bufs=2, space="PSUM"))
    ps_tp = ctx.enter_context(tc.tile_pool(name="ps_tp", bufs=1, space="PSUM"))
    ps_o1 = ctx.enter_context(tc.tile_pool(name="ps_o1", bufs=1, space="PSUM"))
    ps_g = ctx.enter_context(tc.tile_pool(name="ps_g", bufs=1, space="PSUM"))

    # magic constant for the DVE Newton reciprocal (no Ln -> one act table)
    I32 = mybir.dt.int32
    magic = work.tile([8, (BPC // 2) * NT2], I32, tag="magic")
    nc.vector.memset(magic[:], 0x7EF127EA)

    # group-of-16 partition softmax over a [128, nc2] range holding
    # [real, garbage] column pairs in PSUM; returns dense bf16 [128, ncols]
    def softmax(att_pair_view, ncols, tg):
        att_sb = work.tile([128, ncols], F32, tag=tg + "att")
        nc.scalar.activation(att_sb[:].unsqueeze(2), att_pair_view, AF.Copy)
        mask = work.tile([128, ncols], FB, tag=tg + "mask")
        nc.vector.tensor_scalar(mask[:], att_sb[:], 0.0, None, op0=OP.not_equal)
        lr = work.tile([128, ncols], F32, tag=tg + "lr")
        nc.vector.scalar_tensor_tensor(
            lr[:], att_sb[:], 0.01, att_sb[:], op0=OP.mult, op1=OP.max)
        ex = work.tile([128, ncols], FB, tag=tg + "ex")
        nc.scalar.activation(ex[:], lr[:], AF.Exp)
        exm = work.tile([128, ncols], FB, tag=tg + "exm")
        nc.vector.tensor_mul(exm[:], ex[:], mask[:])
        sums_ps = ps_sm.tile([8, ncols], F32, tag="sm_ps")
        nc.tensor.matmul(sums_ps[:], sel16[:], exm[:], start=True, stop=True)
        sums = work.tile([8, ncols], F32, tag=tg + "sumsb")
        nc.vector.tensor_scalar_add(sums[:], sums_ps[:], 1e-30)
        sums_n = work.tile([8, ncols], F32, tag=tg + "sumsn")
        nc.vector.tensor_scalar(
            sums_n[:], sums_ps[:], 1e-30, -1.0, op0=OP.add, op1=OP.mult)
        # rinv = 1/sums: magic-number seed + 2 Newton steps, all on DVE;
        # r' = (t+2)*r with t = (-x)*r keeps stt's (in0 op scalar) order safe
        i0 = work.tile([8, ncols], I32, tag=tg + "i0")
        nc.vector.tensor_sub(i0[:], magic[:, 0:ncols], sums[:].bitcast(I32))
        r0 = i0[:].bitcast(F32)
        t1 = work.tile([8, ncols], F32, tag=tg + "t1")
        nc.vector.tensor_mul(t1[:], sums_n[:], r0)
        r1 = work.tile([8, ncols], F32, tag=tg + "r1")
        nc.vector.scalar_tensor_tensor(
            r1[:], t1[:], 2.0, r0, op0=OP.add, op1=OP.mult)
        t2 = work.tile([8, ncols], F32, tag=tg + "t2")
        nc.vector.tensor_mul(t2[:], sums_n[:], r1[:])
        rinv = work.tile([8, ncols], FB, tag=tg + "rinv")
        nc.vector.scalar_tensor_tensor(
            rinv[:], t2[:], 2.0, r1[:], op0=OP.add, op1=OP.mult)
        rrep_ps = ps_sm.tile([128, ncols], F32, tag="sm_ps")
        nc.tensor.matmul(rrep_ps[:], rep16[:], rinv[:], start=True, stop=True)
        attn = work.tile([128, ncols], F32, tag=tg + "attn")
        nc.vector.tensor_mul(attn[:], exm[:], rrep_ps[:])
        m2 = work.tile([128, ncols], FB, tag=tg + "m2")
        nc.vector.tensor_scalar(m2[:], attn[:], 1.0 / 16.0, None, op0=OP.not_equal)
        attf = work.tile([128, ncols], FB, tag=tg + "attf")
        nc.vector.tensor_mul(attf[:], attn[:], m2[:])
        return attf

    att2_ps = ps_att.tile([128, 2 * BPC * NT2], F32, tag="att2")
    att1_ps = ps_att.tile([128, 2 * BPC * NT1], F32, tag="att1")

    for j in range(BPC):
        k2tile, v2tile = k2tiles[j], v2tiles[j]
        # ---- attention logits ----
        for tt in range(NT2):
            col = 2 * (j * NT2 + tt)
            nc.tensor.matmul(
                att2_ps[:, col:col + 2],
                k2tile[:, tt * 128:(tt + 1) * 128],
                qp["qp2"][:, j:j + 2],
                start=True, stop=True,
            )
        for tt in range(NT1):
            col = 2 * (j * NT1 + tt)
            nc.tensor.matmul(
                att1_ps[:, col:col + 2],
                k1t[:, j * EC + tt * 128: j * EC + (tt + 1) * 128],
                qp["qp1"][:, j:j + 2],
                start=True, stop=True,
            )

        # ---- softmax ----
        a2view = att2_ps[:].rearrange("p (c two) -> p c two", two=2)[
            :, j * NT2:(j + 1) * NT2, 0:1]
        a1view = att1_ps[:].rearrange("p (c two) -> p c two", two=2)[
            :, j * NT1:(j + 1) * NT1, 0:1]
        att2f = softmax(a2view, NT2, "s2_")
        att1f = softmax(a1view, NT1, "s1_")

        # selector builds (0-step broadcast dims; mask picks the diagonal)
        nc.vector.tensor_mul(
            att_sel[:, j * NT2 * 8:(j + 1) * NT2 * 8].rearrange(
                "p (c g) -> p c g", g=8),
            att2f[:].unsqueeze(2).broadcast_to([128, NT2, 8]),
            sel16[:].unsqueeze(1).broadcast_to([128, NT2, 8]),
        )
        nc.vector.tensor_mul(
            sel24[:, j * NT1 * E:(j + 1) * NT1 * E].rearrange(
                "p (t e) -> p t e", t=NT1),
            att1f[:].unsqueeze(2).broadcast_to([128, NT1, E]),
            m24[:].rearrange("p (t e) -> p t e", t=NT1),
        )

        # ---- combined2 (transposed), then PE-transpose to row-major ----
        c2t_ps = ps_c2.tile([KD, EC], F32, tag="c2t")
        for tt in range(NT2):
            nc.tensor.matmul(
                c2t_ps[:, tt * 8:(tt + 1) * 8],
                v2tile[:, tt * KD:(tt + 1) * KD],
                att_sel[:, (j * NT2 + tt) * 8:(j * NT2 + tt + 1) * 8],
                start=True, stop=True,
            )
        c2t = work.tile([KD, EC], FB, tag="c2t_sb")
        nc.vector.tensor_copy(c2t[:], c2t_ps[:])
        c2sb = work.tile([128, NT1 * KD], FB, tag="c2sb")
        for tt in range(NT1):
            tp_ps = ps_tp.tile([128, KD], FB, tag="tp")
            nc.tensor.transpose(tp_ps[:], c2t[:, tt * 128:(tt + 1) * 128], ident[:])
            nc.vector.tensor_copy(c2sb[:, tt * KD:(tt + 1) * KD], tp_ps[:])

        # ---- layer 1: out1 = [sel24.T @ v1 | sel24.T @ c2] ----
        out1_ps = ps_o1.tile([E, OD], F32, tag="out1")
        for tt in range(NT1):
            nc.tensor.matmul(
                out1_ps[:, 0:KD],
                sel24[:, (j * NT1 + tt) * E:(j * NT1 + tt + 1) * E],
                v1r[:, (j * NT1 + tt) * KD:(j * NT1 + tt + 1) * KD],
                start=(tt == 0), stop=(tt == NT1 - 1),
            )
        for tt in range(NT1):
            nc.tensor.matmul(
                out1_ps[:, KD:OD],
                sel24[:, (j * NT1 + tt) * E:(j * NT1 + tt + 1) * E],
                c2sb[:, tt * KD:(tt + 1) * KD],
                start=(tt == 0), stop=(tt == NT1 - 1),
            )
        table = work.tile([E, OD], FB, tag="table")
        nc.vector.tensor_copy(table[:], out1_ps[:])

        g_ps = ps_g.tile([128, OD], F32, tag="gath")
        nc.tensor.matmul(
            g_ps[:], gmat[:, j * 128:(j + 1) * 128], table[:],
            start=True, stop=True,
        )
        osb = work.tile([128, OD], FB, tag="osb")
        nc.vector.tensor_copy(osb[:], g_ps[:])
        nc.sync.dma_start(t["out"][j, :, :], osb[:])


def prep_inputs(inputs: dict) -> list[dict]:
    """Split full inputs into per-core input maps (host-side relayout only)."""
    q = np.ascontiguousarray(inputs["q"][:, 0, :], dtype=np.float32)      # [B, 768]
    k1 = np.asarray(inputs["k1"], dtype=np.float32)
    v1 = np.asarray(inputs["v1"], dtype=np.float32)
    k2 = np.asarray(inputs["k2"], dtype=np.float32)
    v2 = np.asarray(inputs["v2"], dtype=np.float32)
    ent = np.asarray(inputs["input_ent"])

    scale = np.float32(1.0 / math.sqrt(KD))
    wkv2 = np.asarray(inputs["Wkv2"], np.float32) * scale
    wkv1 = np.asarray(inputs["Wkv1"], np.float32) * scale
    wq2t = (np.asarray(inputs["Wq2"], np.float32).T.reshape(NQ, 128, KD)
            .transpose(1, 0, 2).reshape(128, NQ * KD))
    wq1t = (np.asarray(inputs["Wq1"], np.float32).T.reshape(NQ, 128, KD)
            .transpose(1, 0, 2).reshape(128, NQ * KD))
    bqf = np.stack([np.asarray(inputs["bq2"], np.float32),
                    np.asarray(inputs["bq1"], np.float32)], axis=1)  # [KD, 2]

    pp = np.arange(128)
    sel16 = (pp[:, None] // 16 == np.arange(8)[None, :]).astype(np.float32)
    rep16 = np.ascontiguousarray(sel16.T)
    te = np.arange(NT1 * E)
    m24 = (te[None, :] % E == 8 * (te[None, :] // E) + pp[:, None] // 16).astype(np.float32)
    ident = np.eye(KD, dtype=np.float32)

    mask = ent != 0
    rank = np.cumsum(mask, axis=1) - 1

    base = {"q0t": None, "wq2t": wq2t, "wq1t": wq1t, "m24": m24,
            "sel16": sel16, "wkv2": wkv2, "wkv1": wkv1,
            "ident": ident, "rep16": rep16, "gmat": None}

    maps = []
    for i in range(NCORES):
        bs = slice(i * BPC, (i + 1) * BPC)
        k2c, v2c = k2[bs], v2[bs]
        k1c, v1c = k1[bs], v1[bs]
        k2tc = np.ascontiguousarray(
            k2c.reshape(BPC, ROWS2, KD).transpose(0, 2, 1)).astype(BF16NP)
        v2rc = np.ascontiguousarray(
            v2c.reshape(BPC, NT2, 128, KD).transpose(0, 2, 1, 3)
            .reshape(BPC, 128, NT2 * KD)).astype(BF16NP)
        k1tc = np.ascontiguousarray(
            k1c.reshape(BPC, EC, KD).transpose(2, 0, 1)
            .reshape(KD, BPC * EC)).astype(BF16NP)
        v1rc = np.ascontiguousarray(
            v1c.reshape(BPC, NT1, 128, KD).transpose(2, 0, 1, 3)
            .reshape(128, BPC * NT1 * KD)).astype(BF16NP)
        q0tc = (q[bs].T.reshape(NQ, 128, BPC).transpose(1, 0, 2)
                .reshape(128, NQ * BPC))
        gm = np.zeros((E, BPC * 128), np.float32)
        for j in range(BPC):
            b = i * BPC + j
            for s in range(S):
                if mask[b, s]:
                    gm[rank[b, s], j * 128 + s] = 1.0

        cpk = np.zeros((128, CPACK_W), np.float32)
        vals = dict(base)
        vals["q0t"] = q0tc
        vals["gmat"] = gm
        for name, rows, w in CPACK_FIELDS:
            o = CPACK_OFF[name]
            cpk[0:rows, o:o + w] = vals[name]

        maps.append({
            "k2t": k2tc, "v2r": v2rc, "k1t": k1tc, "v1r": v1rc,
            "cpack": cpk.astype(BF16NP), "bqf": bqf,
        })
    return maps


_NC_CACHE = {}


def kernel(**inputs) -> np.ndarray:
    from concourse.bass_utils import run_bass_kernel_spmd

    if "nc" not in _NC_CACHE:
        _NC_CACHE["nc"] = build_nc()
    nc = _NC_CACHE["nc"]
    maps = prep_inputs(inputs)
    res = run_bass_kernel_spmd(nc, maps, list(range(NCORES))).results
    out = np.concatenate([np.asarray(res[i]["out"], dtype=np.float32)
                          for i in range(NCORES)], axis=0)
    return np.ascontiguousarray(out.reshape(B, S, OD))


# revision 30
# speedup vs baseline: 1.1831x; 1.0158x over previous
"""Trainium2 Bass kernel for nn_DKEncoder (scatter_memory).

Math (per batch b, reformulated from the reference):
  qiL  = tanh(q0 @ WqL.T + bqL)                 (L in {2,1}, tiny)
  qpL  = qiL @ (WkvL / sqrt(100))               (fold the 1/sqrt(kd) scale)
  att2 = k2.flat(6144,100) @ qp2                (PE bf16, k2 host-transposed)
  a2   = masked-softmax_d(leaky_relu(att2))     (partition-group softmax)
  c2   = sum_d a2 * v2                          (PE bf16, block-diag selector)
  att1 = k1.flat(384,100) @ qp1
  a1   = masked-softmax_c(leaky_relu(att1))
  out  = sum_c a1 * concat([v1, c2], -1)        (PE bf16, accumulated selector)
  scatter rows to nonzero input_ent positions   (PE bf16, 0/1 gather matmul)

Sharding: pure data parallel, 4 batches per core across 8 cores.
All input-dependent data flows through DRAM parameters, so the program
is compiled once and reused for any inputs.

Perf notes:
- all big tensors stream as bf16 (halves HBM bytes and PE stationary loads)
- softmax divide on DVE (no Ln) so one activation table set covers
  Tanh/Exp/Copy -> no ACT_TABLE_LOAD stalls mid-kernel
- big DMAs spread across sync/scalar/vector/gpsimd queues
- bufs=4 pools let all per-batch loads prefetch up front
"""

import math
from contextlib import ExitStack

import ml_dtypes
import numpy as np

import concourse.bacc as bacc
import concourse.bass as bass
import concourse.mybir as mybir
import concourse.tile as tile

BF16NP = ml_dtypes.bfloat16

B, S, E, C, D, KD, QD = 32, 128, 24, 16, 16, 100, 768
NCORES = 8
BPC = B // NCORES          # batches per core
EC = E * C                 # 384 (e,c) rows
ROWS2 = EC * D             # 6144 (e,c,d) rows
NT2 = ROWS2 // 128         # 48 layer-0 tiles per batch
NT1 = EC // 128            # 3 layer-1 tiles per batch
NQ = QD // 128             # 6 q-chunks
OD = 2 * KD                # 200 output dim
F32 = mybir.dt.float32
BF16 = mybir.dt.bfloat16
AF = mybir.ActivationFunctionType
OP = mybir.AluOpType
FB = BF16

# packed-constants layout (bf16): name -> (rows, width)
CPACK_FIELDS = [
    ("q0t", 128, NQ * BPC),
    ("wq2t", 128, NQ * KD),
    ("wq1t", 128, NQ * KD),
    ("m24", 128, NT1 * E),
    ("sel16", 128, 8),
    ("wkv2", KD, KD),
    ("wkv1", KD, KD),
    ("ident", KD, KD),
    ("rep16", 8, 128),
    ("gmat", E, BPC * 128),
]
CPACK_W = sum(w for _, _, w in CPACK_FIELDS)
CPACK_OFF = {}
_off = 0
for _n, _r, _w in CPACK_FIELDS:
    CPACK_OFF[_n] = _off
    _off += _w


def build_nc() -> bass.Bass:
    nc = bacc.Bacc(None)
    p = lambda name, shape, out=False, dt=F32: nc.declare_dram_parameter(
        name, list(shape), dt, isOutput=out)

    k2t = p("k2t", [BPC, KD, ROWS2], dt=FB)  # per batch: k2 flat transposed
    v2r = p("v2r", [BPC, 128, NT2 * KD], dt=FB)  # per batch: v2 rows tiled
    k1t = p("k1t", [KD, BPC * EC], dt=FB)    # k1 flat transposed
    v1r = p("v1r", [128, BPC * NT1 * KD], dt=FB)  # v1 rows tiled
    cpack = p("cpack", [128, CPACK_W], dt=FB)     # small constants, bf16
    bqf = p("bqf", [KD, 2])                  # biases, f32
    out = p("out", [BPC, 128, OD], out=True, dt=FB)

    with tile.TileContext(nc) as tc, ExitStack() as ctx:
        _body(ctx, tc, nc, locals())
    nc.compile()
    return nc


def _body(ctx, tc, nc, t):
    consts = ctx.enter_context(tc.tile_pool(name="consts", bufs=1))

    cp = consts.tile([128, CPACK_W], FB, tag="cpack")
    nc.scalar.dma_start(cp[:], t["cpack"][:])
    bqf = consts.tile([KD, 2], F32, tag="bqf")
    nc.scalar.dma_start(bqf[:], t["bqf"][:])

    def cc(name):
        rows, w = next((r, w) for n, r, w in CPACK_FIELDS if n == name)
        o = CPACK_OFF[name]
        return cp[0:rows, o:o + w]

    q0t, wq2t, wq1t, m24, sel16 = cc("q0t"), cc("wq2t"), cc("wq1t"), cc("m24"), cc("sel16")
    wkv2, wkv1 = cc("wkv2"), cc("wkv1")
    ident, rep16, gmat = cc("ident"), cc("rep16"), cc("gmat")

    work = ctx.enter_context(tc.tile_pool(name="work", bufs=1))
    k2pool = ctx.enter_context(tc.tile_pool(name="k2t", bufs=4))
    v2pool = ctx.enter_context(tc.tile_pool(name="v2r", bufs=4))

    # one load queue (sync) in first-need order — aggregate DMA bw is a
    # per-core ceiling, so extra queues don't help; keeping the scalar
    # queue to cpack/bqf only means activations never sit behind a
    # blocked dma_start in the scalar sequencer
    k2tiles, v2tiles = [], []
    for j in range(BPC):
        k2tiles.append(k2pool.tile([KD, ROWS2], FB, tag="k2tile", name=f"k2tile{j}"))
        v2tiles.append(v2pool.tile([128, NT2 * KD], FB, tag="v2tile", name=f"v2tile{j}"))
    k1t = consts.tile([KD, BPC * EC], FB, tag="k1t")
    v1r = consts.tile([128, BPC * NT1 * KD], FB, tag="v1r")
    nc.sync.dma_start(k2tiles[0][:], t["k2t"][0, :, :])
    nc.sync.dma_start(k1t[:], t["k1t"][:])
    nc.sync.dma_start(v1r[:], t["v1r"][:])
    nc.sync.dma_start(v2tiles[0][:], t["v2r"][0, :, :])
    nc.sync.dma_start(k2tiles[1][:], t["k2t"][1, :, :])
    nc.sync.dma_start(v2tiles[1][:], t["v2r"][1, :, :])
    nc.sync.dma_start(k2tiles[2][:], t["k2t"][2, :, :])
    nc.sync.dma_start(v2tiles[2][:], t["v2r"][2, :, :])
    nc.sync.dma_start(k2tiles[3][:], t["k2t"][3, :, :])
    # last v2 tile in halves so batch 3's c2 loop can chase the stream
    VH = NT2 * KD // 2
    nc.sync.dma_start(v2tiles[3][:, 0:VH], t["v2r"][3, :, 0:VH])
    nc.sync.dma_start(v2tiles[3][:, VH:2 * VH], t["v2r"][3, :, VH:2 * VH])

    # ---- Phase Q: qp2/qp1 [100, BPC+1] (zero pad col) ----
    qp = {}
    with tc.tile_pool(name="ps_q", bufs=2, space="PSUM") as ps_q:
        for lname, wqt, wkv, bqcol in (("qp2", wq2t, wkv2, 0), ("qp1", wq1t, wkv1, 1)):
            qtmp = ps_q.tile([KD, BPC], F32, tag="qtmp")
            for c in range(NQ):
                nc.tensor.matmul(
                    qtmp[:],
                    wqt[:, c * KD:(c + 1) * KD],
                    q0t[:, c * BPC:(c + 1) * BPC],
                    start=(c == 0), stop=(c == NQ - 1),
                )
            qi = work.tile([KD, BPC], FB, tag="qi")
            nc.scalar.activation(qi[:], qtmp[:], AF.Tanh,
                                 bias=bqf[:, bqcol:bqcol + 1], scale=1.0)
            qps = ps_q.tile([KD, BPC], F32, tag="qps")
            nc.tensor.matmul(qps[:], wkv[:], qi[:], start=True, stop=True)
            qsb = work.tile([KD, BPC + 1], FB, tag=lname)
            nc.vector.tensor_copy(qsb[:, 0:BPC], qps[:])
            nc.vector.memset(qsb[:, BPC:BPC + 1], 0.0)
            qp[lname] = qsb

    att_sel = work.tile([128, BPC * NT2 * 8], FB, tag="att_sel")
    sel24 = work.tile([128, BPC * NT1 * E], FB, tag="sel24")

    ps_att = ctx.enter_context(tc.tile_pool(name="ps_att", bufs=1, space="PSUM"))
    ps_sm = ctx.enter_context(tc.tile_pool(name="ps_sm", bufs=1, space="PSUM"))
    ps_c2 = ctx.enter_context(tc.tile_pool(name="ps_c2", bufs=2, space="PSUM"))
    ps_tp = ctx.enter_context(tc.tile_pool(name="ps_tp", bufs=1, space="PSUM"))
    ps_o1 = ctx.enter_context(tc.tile_pool(name="ps_o1", bufs=1, space="PSUM"))
    ps_g = ctx.enter_context(tc.tile_pool(name="ps_g", bufs=1, space="PSUM"))

    # magic constant for the DVE Newton reciprocal (no Ln -> one act table)
    I32 = mybir.dt.int32
    magic = work.tile([8, (BPC // 2) * NT2], I32, tag="magic")
    nc.vector.memset(magic[:], 0x7EF127EA)

    # group-of-16 partition softmax over a [128, nc2] range holding
    # [real, garbage] column pairs in PSUM; returns dense bf16 [128, ncols]
    def softmax(att_pair_view, ncols, tg):
        att_sb = work.tile([128, ncols], F32, tag=tg + "att")
        nc.scalar.activation(att_sb[:].unsqueeze(2), att_pair_view, AF.Copy)
        mask = work.tile([128, ncols], FB, tag=tg + "mask")
        nc.vector.tensor_scalar(mask[:], att_sb[:], 0.0, None, op0=OP.not_equal)
        lr = work.tile([128, ncols], F32, tag=tg + "lr")
        nc.vector.scalar_tensor_tensor(
            lr[:], att_sb[:], 0.01, att_sb[:], op0=OP.mult, op1=OP.max)
        ex = work.tile([128, ncols], FB, tag=tg + "ex")
        nc.scalar.activation(ex[:], lr[:], AF.Exp)
        exm = work.tile([128, ncols], FB, tag=tg + "exm")
        nc.vector.tensor_mul(exm[:], ex[:], mask[:])
        sums_ps = ps_sm.tile([8, ncols], F32, tag="sm_ps")
        nc.tensor.matmul(sums_ps[:], sel16[:], exm[:], start=True, stop=True)
        sums = work.tile([8, ncols], F32, tag=tg + "sumsb")
        nc.vector.tensor_scalar_add(sums[:], sums_ps[:], 1e-30)
        sums_n = work.tile([8, ncols], F32, tag=tg + "sumsn")
        nc.vector.tensor_scalar(
            sums_n[:], sums_ps[:], 1e-30, -1.0, op0=OP.add, op1=OP.mult)
        # rinv = 1/sums: magic-number seed + 2 Newton steps, all on DVE;
        # r' = (t+2)*r with t = (-x)*r keeps stt's (in0 op scalar) order safe
        i0 = work.tile([8, ncols], I32, tag=tg + "i0")
        nc.vector.tensor_sub(i0[:], magic[:, 0:ncols], sums[:].bitcast(I32))
        r0 = i0[:].bitcast(F32)
        t1 = work.tile([8, ncols], F32, tag=tg + "t1")
        nc.vector.tensor_mul(t1[:], sums_n[:], r0)
        r1 = work.tile([8, ncols], F32, tag=tg + "r1")
        nc.vector.scalar_tensor_tensor(
            r1[:], t1[:], 2.0, r0, op0=OP.add, op1=OP.mult)
        t2 = work.tile([8, ncols], F32, tag=tg + "t2")
        nc.vector.tensor_mul(t2[:], sums_n[:], r1[:])
        rinv = work.tile([8, ncols], FB, tag=tg + "rinv")
        nc.vector.scalar_tensor_tensor(
            rinv[:], t2[:], 2.0, r1[:], op0=OP.add, op1=OP.mult)
        rrep_ps = ps_sm.tile([128, ncols], F32, tag="sm_ps")
        nc.tensor.matmul(rrep_ps[:], rep16[:], rinv[:], start=True, stop=True)
        attn = work.tile([128, ncols], F32, tag=tg + "attn")
        nc.vector.tensor_mul(attn[:], exm[:], rrep_ps[:])
        m2 = work.tile([128, ncols], FB, tag=tg + "m2")
        nc.vector.tensor_scalar(m2[:], attn[:], 1.0 / 16.0, None, op0=OP.not_equal)
        attf = work.tile([128, ncols], FB, tag=tg + "attf")
        nc.vector.tensor_mul(attf[:], attn[:], m2[:])
        return attf

    att2_ps = ps_att.tile([128, 2 * BPC * NT2], F32, tag="att2")
    att1_ps = ps_att.tile([128, 2 * BPC * NT1], F32, tag="att1")

    for j in range(BPC):
        k2tile, v2tile = k2tiles[j], v2tiles[j]
        # ---- attention logits ----
        for tt in range(NT2):
            col = 2 * (j * NT2 + tt)
            nc.tensor.matmul(
                att2_ps[:, col:col + 2],
                k2tile[:, tt * 128:(tt + 1) * 128],
                qp["qp2"][:, j:j + 2],
                start=True, stop=True,
            )
        for tt in range(NT1):
            col = 2 * (j * NT1 + tt)
            nc.tensor.matmul(
                att1_ps[:, col:col + 2],
                k1t[:, j * EC + tt * 128: j * EC + (tt + 1) * 128],
                qp["qp1"][:, j:j + 2],
                start=True, stop=True,
            )

        # ---- softmax ----
        a2view = att2_ps[:].rearrange("p (c two) -> p c two", two=2)[
            :, j * NT2:(j + 1) * NT2, 0:1]
        a1view = att1_ps[:].rearrange("p (c two) -> p c two", two=2)[
            :, j * NT1:(j + 1) * NT1, 0:1]
        att2f = softmax(a2view, NT2, "s2_")
        att1f = softmax(a1view, NT1, "s1_")

        # selector builds (0-step broadcast dims; mask picks the diagonal)
        nc.vector.tensor_mul(
            att_sel[:, j * NT2 * 8:(j + 1) * NT2 * 8].rearrange(
                "p (c g) -> p c g", g=8),
            att2f[:].unsqueeze(2).broadcast_to([128, NT2, 8]),
            sel16[:].unsqueeze(1).broadcast_to([128, NT2, 8]),
        )
        nc.vector.tensor_mul(
            sel24[:, j * NT1 * E:(j + 1) * NT1 * E].rearrange(
                "p (t e) -> p t e", t=NT1),
            att1f[:].unsqueeze(2).broadcast_to([128, NT1, E]),
            m24[:].rearrange("p (t e) -> p t e", t=NT1),
        )

        # ---- combined2 (transposed), then PE-transpose to row-major ----
        c2t_ps = ps_c2.tile([KD, EC], F32, tag="c2t")
        for tt in range(NT2):
            nc.tensor.matmul(
                c2t_ps[:, tt * 8:(tt + 1) * 8],
                v2tile[:, tt * KD:(tt + 1) * KD],
                att_sel[:, (j * NT2 + tt) * 8:(j * NT2 + tt + 1) * 8],
                start=True, stop=True,
            )
        c2t = work.tile([KD, EC], FB, tag="c2t_sb")
        nc.vector.tensor_copy(c2t[:], c2t_ps[:])
        c2sb = work.tile([128, NT1 * KD], FB, tag="c2sb")
        for tt in range(NT1):
            tp_ps = ps_tp.tile([128, KD], FB, tag="tp")
            nc.tensor.transpose(tp_ps[:], c2t[:, tt * 128:(tt + 1) * 128], ident[:])
            nc.vector.tensor_copy(c2sb[:, tt * KD:(tt + 1) * KD], tp_ps[:])

        # ---- layer 1: out1 = [sel24.T @ v1 | sel24.T @ c2] ----
        out1_ps = ps_o1.tile([E, OD], F32, tag="out1")
        for tt in range(NT1):
            nc.tensor.matmul(
                out1_ps[:, 0:KD],
                sel24[:, (j * NT1 + tt) * E:(j * NT1 + tt + 1) * E],
                v1r[:, (j * NT1 + tt) * KD:(j * NT1 + tt + 1) * KD],
                start=(tt == 0), stop=(tt == NT1 - 1),
            )
        for tt in range(NT1):
            nc.tensor.matmul(
                out1_ps[:, KD:OD],
                sel24[:, (j * NT1 + tt) * E:(j * NT1 + tt + 1) * E],
                c2sb[:, tt * KD:(tt + 1) * KD],
                start=(tt == 0), stop=(tt == NT1 - 1),
            )
        table = work.tile([E, OD], FB, tag="table")
        nc.vector.tensor_copy(table[:], out1_ps[:])

        g_ps = ps_g.tile([128, OD], F32, tag="gath")
        nc.tensor.matmul(
            g_ps[:], gmat[:, j * 128:(j + 1) * 128], table[:],
            start=True, stop=True,
        )
        osb = work.tile([128, OD], FB, tag="osb")
        nc.vector.tensor_copy(osb[:], g_ps[:])
        nc.scalar.dma_start(t["out"][j, :, :], osb[:])


def prep_inputs(inputs: dict) -> list[dict]:
    """Split full inputs into per-core input maps (host-side relayout only)."""
    q = np.ascontiguousarray(inputs["q"][:, 0, :], dtype=np.float32)      # [B, 768]
    k1 = np.asarray(inputs["k1"], dtype=np.float32)
    v1 = np.asarray(inputs["v1"], dtype=np.float32)
    k2 = np.asarray(inputs["k2"], dtype=np.float32)
    v2 = np.asarray(inputs["v2"], dtype=np.float32)
    ent = np.asarray(inputs["input_ent"])

    scale = np.float32(1.0 / math.sqrt(KD))
    wkv2 = np.asarray(inputs["Wkv2"], np.float32) * scale
    wkv1 = np.asarray(inputs["Wkv1"], np.float32) * scale
    wq2t = (np.asarray(inputs["Wq2"], np.float32).T.reshape(NQ, 128, KD)
            .transpose(1, 0, 2).reshape(128, NQ * KD))
    wq1t = (np.asarray(inputs["Wq1"], np.float32).T.reshape(NQ, 128, KD)
            .transpose(1, 0, 2).reshape(128, NQ * KD))
    bqf = np.stack([np.asarray(inputs["bq2"], np.float32),
                    np.asarray(inputs["bq1"], np.float32)], axis=1)  # [KD, 2]

    pp = np.arange(128)
    sel16 = (pp[:, None] // 16 == np.arange(8)[None, :]).astype(np.float32)
    rep16 = np.ascontiguousarray(sel16.T)
    te = np.arange(NT1 * E)
    m24 = (te[None, :] % E == 8 * (te[None, :] // E) + pp[:, None] // 16).astype(np.float32)
    ident = np.eye(KD, dtype=np.float32)

    mask = ent != 0
    rank = np.cumsum(mask, axis=1) - 1

    base = {"q0t": None, "wq2t": wq2t, "wq1t": wq1t, "m24": m24,
            "sel16": sel16, "wkv2": wkv2, "wkv1": wkv1,
            "ident": ident, "rep16": rep16, "gmat": None}

    maps = []
    for i in range(NCORES):
        bs = slice(i * BPC, (i + 1) * BPC)
        k2c, v2c = k2[bs], v2[bs]
        k1c, v1c = k1[bs], v1[bs]
        k2tc = np.ascontiguousarray(
            k2c.reshape(BPC, ROWS2, KD).transpose(0, 2, 1)).astype(BF16NP)
        v2rc = np.ascontiguousarray(
            v2c.reshape(BPC, NT2, 128, KD).transpose(0, 2, 1, 3)
            .reshape(BPC, 128, NT2 * KD)).astype(BF16NP)
        k1tc = np.ascontiguousarray(
            k1c.reshape(BPC, EC, KD).transpose(2, 0, 1)
            .reshape(KD, BPC * EC)).astype(BF16NP)
        v1rc = np.ascontiguousarray(
            v1c.reshape(BPC, NT1, 128, KD).transpose(2, 0, 1, 3)
            .reshape(128, BPC * NT1 * KD)).astype(BF16NP)
        q0tc = (q[bs].T.reshape(NQ, 128, BPC).transpose(1, 0, 2)
                .reshape(128, NQ * BPC))
        gm = np.zeros((E, BPC * 128), np.float32)
        for j in range(BPC):
            b = i * BPC + j
            for s in range(S):
                if mask[b, s]:
                    gm[rank[b, s], j * 128 + s] = 1.0

        cpk = np.zeros((128, CPACK_W), np.float32)
        vals = dict(base)
        vals["q0t"] = q0tc
        vals["gmat"] = gm
        for name, rows, w in CPACK_FIELDS:
            o = CPACK_OFF[name]
            cpk[0:rows, o:o + w] = vals[name]

        maps.append({
            "k2t": k2tc, "v2r": v2rc, "k1t": k1tc, "v1r": v1rc,
            "cpack": cpk.astype(BF16NP), "bqf": bqf,
        })
    return maps


_NC_CACHE = {}


def kernel(**inputs) -> np.ndarray:
    from concourse.bass_utils import run_bass_kernel_spmd

    if "nc" not in _NC_CACHE:
        _NC_CACHE["nc"] = build_nc()
    nc = _NC_CACHE["nc"]
    maps = prep_inputs(inputs)
    res = run_bass_kernel_spmd(nc, maps, list(range(NCORES))).results
    out = np.concatenate([np.asarray(res[i]["out"], dtype=np.float32)
                          for i in range(NCORES)], axis=0)
    return np.ascontiguousarray(out.reshape(B, S, OD))
